# revision 15
# baseline (speedup 1.0000x reference)
"""GNN message-passing layer (LplsNorm + residual conv) on 8 Trainium2 cores.

Computation (reference, all f32):
    degree = A.sum(-1); ds = degree**-0.5
    mf  = f + ds[:,None] * (A @ (ds[:,None] * f))      # a_norm = ds A ds
    out = relu(mf @ W + b)

Distribution: A row-sharded over 8 cores ([1024, 8192] each), feature
replicated (each core reads full f from its own HBM copy).

Per-core schedule (v8):
  Identity used: out = relu(dsr * (mf2 @ W) + b) with
    mf2 = Y/64 + fres * sqrt(degree),  Y = A @ ((64*ds) * f)
  (row-scale dsr commutes past @W; x64 keeps fp8 xp in normal range).
  The message term is ~1.3% of the output magnitude, so A/xp in fp8e4
  (DoubleRow, 2 k-chunks per matmul) costs ~0.05% output error.

  Phase 1 (DMA-bound): stream A in [128, 4096] f32 chunks (2 MiB DMAs
    alternating over both HWDGE rings); ScalarE casts to bf16 + row-sum
    accum (degree); PE transposes all tiles (kc-major at_res, fp8e4,
    resident 64 KiB/partition); fres is scaled by sqrt(degree) and
    PE-transposed into fT2 per m-tile. The ds AllGather is split into
    FOUR quarter-gathers fired at mt=1,3,5,7: cross-core skew makes a
    gather complete only ~20us after the SLOWEST core reaches that mt,
    so quarter k's ds arrives just as the matmuls for quarter k-1
    drain. cout readbacks ride the gpsimd ring behind their own AG.
  Phase 3 (PE-bound): Y^T accumulation via fp8 DoubleRow: per kc-PAIR
    one xp2 = (64*ds)*f fp8 tile; 4 stationary xp-quarters x 2 moving
    at-halves ([128,2,512] APs) into 8 PSUM banks; f streamed once.
    Each quarter's ds block-transposes are emitted just before its
    pair loop (DVE FIFO stays unblocked). Epilogue: mf2T = Y^T/64 +
    fT2 (DVE/GPSIMD stt, in place), mf2T @ W (32 bf16 matmuls),
    dsr-scale + bias via DVE stt, relu, out DMA.
"""

import numpy as np

import concourse.bass as bass
import concourse.mybir as mybir
import concourse.tile as tile
from concourse import bacc
from concourse import bass_utils
from concourse.masks import make_identity

N = 8192
D = 512
NCORES = 8
P = 128
R = N // NCORES          # rows per core: 1024
MT = R // P              # m-tiles per core: 8
KC = N // P              # k-chunks: 64
ACH = 4096               # A stream chunk width (f32 -> 2 MiB per DMA)
NACH = N // ACH          # stream chunks per m-tile: 2
GPC = ACH // (4 * P)     # transpose groups (of 4 tiles) per stream chunk: 8
NQ = 4                   # ds AllGather quarters
XPS = 64.0               # fp8 xp pre-scale (~1/ds), divided back out later

F32 = mybir.dt.float32
BF16 = mybir.dt.bfloat16
FP8 = mybir.dt.float8e4

_NC_CACHE = {}


def _build():
    nc = bacc.Bacc("TRN2", target_bir_lowering=False, debug=False, num_devices=NCORES)

    a_d = nc.dram_tensor("a", [R, N], F32, kind="ExternalInput")
    f_d = nc.dram_tensor("f", [N, D], F32, kind="ExternalInput")
    fres_d = nc.dram_tensor("fres", [R, D], F32, kind="ExternalInput")
    w_d = nc.dram_tensor("w", [D, D], F32, kind="ExternalInput")
    b_d = nc.dram_tensor("bias", [1, D], F32, kind="ExternalInput")
    out_d = nc.dram_tensor("out", [R, D], F32, kind="ExternalOutput")

    AX = mybir.AxisListType.X
    ALU = mybir.AluOpType
    ACT = mybir.ActivationFunctionType
    DR = mybir.MatmulPerfMode.DoubleRow

    with tile.TileContext(nc) as tc:
        with (
            tc.tile_pool(name="const", bufs=1) as constp,
            tc.tile_pool(name="deg", bufs=1) as degp,
            tc.tile_pool(name="astream", bufs=3) as astreamp,
            tc.tile_pool(name="small", bufs=2) as smallp,
            tc.tile_pool(name="atres", bufs=1) as atresp,
            tc.tile_pool(name="ft2", bufs=1) as ft2p,
            tc.tile_pool(name="fstream", bufs=3) as fstreamp,
            tc.tile_pool(name="xpp", bufs=4) as xpp,
            tc.tile_pool(name="epi", bufs=2) as epip,
            tc.tile_pool(name="dram", bufs=1, space="DRAM") as dramp,
        ):
            # ---- constants ----
            identity = constp.tile([P, P], F32)
            make_identity(nc, identity[:])
            identity_bf = constp.tile([P, P], BF16)
            make_identity(nc, identity_bf[:])
            ones1 = constp.tile([1, P], F32)
            nc.gpsimd.memset(ones1[:], 1.0)
            b_sb = constp.tile([1, D], F32)
            nc.scalar.dma_start(b_sb[:], b_d.ap())
            # W: load f32 (staged in fstream pool), cast to bf16 blocks:
            # w_bf[:, q*D:(q+1)*D] = W[q*128:(q+1)*128, :]
            w_bf = constp.tile([P, 4 * D], BF16)
            for wi in range(2):
                wstage = fstreamp.tile([P, 2 * D], F32, tag="fch", bufs=4, name=f"wst{wi}")
                for q2 in range(2):
                    q = wi * 2 + q2
                    nc.scalar.dma_start(
                        wstage[:, q2 * D : (q2 + 1) * D],
                        w_d.ap()[q * P : (q + 1) * P, :],
                    )
                nc.vector.tensor_copy(
                    w_bf[:, wi * 2 * D : (wi * 2 + 2) * D], wstage[:]
                )

            # resident transposed-A store, kc-major: tile (kc, mt) at
            # cols kc*(MT*P) + mt*P
            at_res = atresp.tile([P, KC * MT * P], FP8)
            at3 = at_res[:].rearrange("p (kc m) -> p kc m", kc=KC)
            # fT2[q] cols mt*P.. = (fres * sqrt(degree))^T for feature block q
            ft2 = [ft2p.tile([P, MT * P], BF16, name=f"ft2_{q}") for q in range(4)]

            # collective buffers (4 quarters; cin rows 2-3 are junk padding
            # so cout keeps 32x32-transposable shape)
            cins = [dramp.tile([4, P], F32, name=f"cin{h}") for h in range(NQ)]
            couts = [
                dramp.tile([NCORES * 4, P], F32, name=f"cout{h}") for h in range(NQ)
            ]

            degree_sb = degp.tile([P, MT], F32)  # col mt = degree of rows mt*128..
            dsown = degp.tile([P, MT], F32)      # degree**-0.5 (own rows)
            ds_sb = degp.tile([P, KC], F32)      # ds_sb[p, kc] = XPS*ds[kc*128+p]
            ds4 = ds_sb[:].rearrange("p (d k) -> p d k", k=MT)
            dsscr = degp.tile([P, 32], F32)      # transpose scratch
            dscols = [
                smallp.tile(
                    [NCORES * 4, P], F32, tag=f"dscol{h}", bufs=1, name=f"dscol{h}"
                )
                for h in range(NQ)
            ]

            def emit_ds_quarter(h):
                # cout_h rows d*4+j (j<2 valid) -> scratch[p, d*4+j] -> ds_sb
                for a in range(4):
                    nc.vector.transpose(
                        dsscr[32 * a : 32 * (a + 1), :].rearrange(
                            "p (x y) -> p x y", y=4
                        ),
                        dscols[h][0:32, 32 * a : 32 * (a + 1)].rearrange(
                            "r (x y) -> r x y", y=4
                        ),
                    )
                # pick valid rows (j<2): ds = sqrt(XPS^2 / degree)
                scr3 = dsscr[:].rearrange("p (d j) -> p d j", j=4)
                rcp = degp.tile([P, 16], F32, tag="rcp", name=f"rcp{h}")
                nc.vector.reciprocal(
                    rcp[:].rearrange("p (d j) -> p d j", j=2), scr3[:, :, 0:2]
                )
                nc.scalar.activation(
                    ds4[:, :, 2 * h : 2 * h + 2],
                    rcp[:].rearrange("p (d j) -> p d j", j=2),
                    ACT.Sqrt,
                    scale=XPS * XPS,
                )

            with (
                tc.tile_pool(name="psA", bufs=3, space="PSUM") as psA,
                tc.tile_pool(name="psaux", bufs=2, space="PSUM") as psaux,
            ):
                # bias broadcast [128, D] via K=1 matmul (once)
                b_ps = psA.tile([P, D], F32, tag="trp", name="b_ps")
                nc.tensor.matmul(b_ps[:], ones1[:], b_sb[:])
                b_bcast = constp.tile([P, D], F32)
                nc.vector.tensor_copy(b_bcast[:], b_ps[:])

                # ---- phase 1: stream A, degree + transpose (A read once) ----
                for mt in range(MT):
                    dcols = smallp.tile([P, NACH], F32, tag="dcols")
                    for c in range(NACH):
                        ach = astreamp.tile([P, ACH], F32, tag="ach", bufs=3)
                        nc.sync.dma_start(
                            ach[:],
                            a_d.ap()[mt * P : (mt + 1) * P, c * ACH : (c + 1) * ACH],
                        )
                        achb = astreamp.tile([P, ACH], BF16, tag="achb", bufs=2)
                        nc.scalar.activation(
                            achb[:], ach[:], ACT.Copy, accum_out=dcols[:, c : c + 1]
                        )
                        for g in range(GPC):
                            trp = psA.tile([P, 4 * P], F32, tag="trp")
                            for q in range(4):
                                nc.tensor.matmul(
                                    trp[:, q * P : (q + 1) * P],
                                    achb[:, (g * 4 + q) * P : (g * 4 + q + 1) * P],
                                    identity_bf[:],
                                )
                            kc0 = c * (GPC * 4) + g * 4
                            nc.vector.tensor_copy(
                                at3[:, kc0 : kc0 + 4, mt * P : (mt + 1) * P],
                                trp[:].rearrange("p (a b) -> p a b", a=4),
                            )
                    # degree[:, mt] = sum(dcols) -- on ScalarE (keeps the
                    # congested DVE queue out of the AllGather chain)
                    dred = smallp.tile([P, NACH], F32, tag="dred")
                    nc.scalar.activation(
                        dred[:], dcols[:], ACT.Copy,
                        accum_out=degree_sb[:, mt : mt + 1],
                    )
                    # fire a ds AllGather quarter at mt = 1, 3, 5, 7; the whole
                    # chain is ACT+PE+sync only; the cout readback rides gpsimd
                    # right behind its own AG
                    if mt % 2 == 1:
                        h = mt // 2
                        hs = 2 * h
                        # gather RAW degrees; ds math happens on the receive
                        # side in phase 3 where ACT/DVE are idle
                        dsT_ps = psaux.tile([2, P], F32, tag="aux", name=f"dsT{h}")
                        nc.tensor.matmul(
                            dsT_ps[:], degree_sb[:, hs : hs + 2], identity[:]
                        )
                        dsT_sb = smallp.tile([4, P], F32, tag="degT")
                        nc.scalar.activation(dsT_sb[0:2, :], dsT_ps[:], ACT.Copy)
                        nc.sync.dma_start(cins[h][:], dsT_sb[:])
                        nc.gpsimd.collective_compute(
                            "AllGather",
                            ALU.bypass,
                            ins=[cins[h].opt()],
                            outs=[couts[h].opt()],
                            replica_groups=[list(range(NCORES))],
                        )
                        nc.gpsimd.dma_start(dscols[h][:], couts[h][:])

                # fres * sqrt(degree), transposed into fT2 (after the stream:
                # interleaving this into the chunk loop knotted the engine
                # FIFOs and stalled the stream for ~15us). One batched SQRT
                # (a table reload costs 1.3us on ACT); dsown = 1/sqd.
                sqd8 = degp.tile([P, MT], F32)
                nc.scalar.activation(sqd8[:], degree_sb[:], ACT.Sqrt)
                nc.vector.reciprocal(dsown[:], sqd8[:])
                for mt in range(MT):
                    fres_t = epip.tile([P, D], F32, tag="fres", bufs=2)
                    nc.scalar.dma_start(
                        fres_t[:], fres_d.ap()[mt * P : (mt + 1) * P, :]
                    )
                    fres2 = epip.tile([P, D], BF16, tag="fres2")
                    nc.scalar.activation(
                        fres2[:], fres_t[:], ACT.Copy, scale=sqd8[:, mt : mt + 1]
                    )
                    for q in range(4):
                        fT_ps = psaux.tile([P, P], F32, tag="aux")
                        nc.tensor.matmul(
                            fT_ps[:],
                            fres2[:, q * P : (q + 1) * P],
                            identity_bf[:],
                        )
                        nc.scalar.activation(
                            ft2[q][:, mt * P : (mt + 1) * P], fT_ps[:], ACT.Copy
                        )

            # ---- phase 3: Y^T accumulation, fp8 DoubleRow, 8 PSUM banks ----
            f2_blk = f_d.ap().rearrange("(a c p) d -> a p c d", c=2, p=P)
            with tc.tile_pool(name="psY", bufs=8, space="PSUM") as psY:
                ys = [
                    psY.tile([P, D], F32, tag="y", name=f"yt{q}_{hh}")
                    for q in range(4)
                    for hh in range(2)
                ]
                kp = 0
                for h in range(NQ):
                    # quarter h's ds lands while quarter h-1's matmuls drain.
                    # tile_wait_until pins these DVE ops late in the scheduled
                    # queue so the (optimistically modeled) collective cannot
                    # pull them ahead of phase-1 stream work and stall it.
                    with tc.tile_wait_until(0.15 + 0.012 * h):
                        emit_ds_quarter(h)
                    for d8 in range(NCORES):
                        kc0 = d8 * 8 + 2 * h
                        fch = fstreamp.tile([P, 2 * D], F32, tag="fch", bufs=4)
                        fring = nc.sync if d8 % 2 == 0 else nc.scalar
                        fring.dma_start(
                            fch[:].rearrange("p (c d) -> p c d", c=2),
                            f2_blk[kc0 // 2],
                        )
                        xp2 = xpp.tile([P, 2 * D], FP8, tag="xp")
                        for j in range(2):
                            nc.vector.tensor_scalar_mul(
                                xp2[:, j * D : (j + 1) * D],
                                fch[:, j * D : (j + 1) * D],
                                ds_sb[:, kc0 + j : kc0 + j + 1],
                            )
                        xp3 = xp2[:].rearrange("p (c d) -> p c d", c=2)
                        for q in range(4):
                            for hh in range(2):
                                nc.tensor.matmul(
                                    ys[q * 2 + hh][:],
                                    xp3[:, :, q * P : (q + 1) * P],
                                    at3[:, kc0 : kc0 + 2, hh * D : (hh + 1) * D],
                                    start=(kp == 0),
                                    stop=(kp == KC // 2 - 1),
                                    perf_mode=DR,
                                )
                        kp += 1

                # mf2T = Y^T/XPS + fT2 (in place; hh=0 on DVE first so the
                # epilogue for m-tiles 0-3 can start, hh=1 on gpsimd)
                for q in range(4):
                    nc.vector.scalar_tensor_tensor(
                        ft2[q][:, 0:D],
                        ys[q * 2 + 0][:],
                        1.0 / XPS,
                        ft2[q][:, 0:D],
                        op0=ALU.mult,
                        op1=ALU.add,
                    )
                for q in range(4):
                    nc.vector.scalar_tensor_tensor(
                        ft2[q][:, D : 2 * D],
                        ys[q * 2 + 1][:],
                        1.0 / XPS,
                        ft2[q][:, D : 2 * D],
                        op0=ALU.mult,
                        op1=ALU.add,
                    )

                # ---- epilogue: out = relu(dsr * (mf2T.T @ W) + b) ----
                # o_ps reuses the psY ring (slot mt frees after its stt)
                for mt in range(MT):
                    o_ps = psY.tile([P, D], F32, tag="y", name=f"o{mt}")
                    for q in range(4):
                        nc.tensor.matmul(
                            o_ps[:],
                            ft2[q][:, mt * P : (mt + 1) * P],
                            w_bf[:, q * D : (q + 1) * D],
                            start=(q == 0),
                            stop=(q == 3),
                        )
                    opre = epip.tile([P, D], F32, tag="opre", bufs=2)
                    nc.vector.scalar_tensor_tensor(
                        opre[:],
                        o_ps[:],
                        dsown[:, mt : mt + 1],
                        b_bcast[:],
                        op0=ALU.mult,
                        op1=ALU.add,
                    )
                    osb = epip.tile([P, D], F32, tag="osb", bufs=2)
                    nc.scalar.activation(osb[:], opre[:], ACT.Relu)
                    nc.scalar.dma_start(
                        out_d.ap()[mt * P : (mt + 1) * P, :], osb[:]
                    )

    nc.compile()
    return nc


def _get_nc():
    if "nc" not in _NC_CACHE:
        _NC_CACHE["nc"] = _build()
    return _NC_CACHE["nc"]


def run(inputs, trace=False, trace_kwargs=None):
    """Run the SPMD kernel; returns (full_output, BassKernelResults)."""
    a = np.ascontiguousarray(np.asarray(inputs["adjacency_matrix"], dtype=np.float32))
    f = np.ascontiguousarray(np.asarray(inputs["feature"], dtype=np.float32))
    w = np.ascontiguousarray(np.asarray(inputs["W"], dtype=np.float32))
    b = np.ascontiguousarray(np.asarray(inputs["b"], dtype=np.float32)).reshape(1, D)

    nc = _get_nc()
    in_maps = []
    for d in range(NCORES):
        rows = slice(d * R, (d + 1) * R)
        in_maps.append({"a": a[rows], "f": f, "fres": f[rows], "w": w, "bias": b})
    res = bass_utils.run_bass_kernel_spmd(
        nc,
        in_maps,
        core_ids=list(range(NCORES)),
        trace=trace,
        **(trace_kwargs or {}),
    )
    out = np.concatenate([r["out"] for r in res.results], axis=0)
    return out, res


def kernel(**inputs):
    out, _ = run(inputs, trace=False)
    return out
